# revision 29
# baseline (speedup 1.0000x reference)
"""Trainium2 Bass kernel for batched per-item GRU cell.

Problem: nn_GRU_Cell — B=16, N=207 independent items, each with its own
C=64 -> 3H=192 weight matrices (Wx, Wh).  All ops are per-(b,n):

    xW          = x @ Wx                      [1, 192]
    r           = sigmoid(xW_r + h @ Wh_r + b_r)
    z           = sigmoid(xW_z + h @ Wh_z + b_z)
    hc          = tanh  (xW_c + (r*h) @ Wh_c + b_c)
    h_new       = (1 - z) * h + z * hc

Strategy (per core, items sharded 3312 -> 8 x 414):
  * Weights dominate DMA (bf16: ~20.6MB/core).  Stream once, chunked.
  * Per item, weights are the PE *stationary* operand, K-stacked:
      S_rz = [Wx[:, 0:128] ; Wh[:, 0:128]]  (K=128, M=128)
      S_c  = [Wx[:,128:192]; Wh[:,128:192]] (K=128, M=64)
    moving operand is one bf16 column per item:
      rz-pass: [x ; h]    -> psum_rz[:, item]
      c-pass : [x ; r*h]  -> psum_c[0:64, item]   (folds xW_c in)
  * bf16 weights + moving columns: single-instruction LDW/MM (no
    fp32 LOW/HIGH doubling), FWL weight loads, half the DMA bytes.
    Max rel err vs fp32 reference ~1.8e-3.
  * x/h/bias arrive host-pre-transposed to [j, items]: no on-chip
    transposes; they're loaded once into persistent SBUF tiles.
  * Software pipeline: chunk k's rz matmuls are traced before chunk
    k-1's c matmuls, so the PE always has independent work while the
    DVE/ACT epilogue of the previous chunk produces r*h.
  * Chunk sizes ramp up (first w-DMA gates the first matmul) and the
    last chunk is short (post-PE drain tail).
"""

import numpy as np

import concourse.bass as bass
import concourse.mybir as mybir
import concourse.tile as tile
from concourse import bacc
from concourse.bass_utils import run_bass_kernel_spmd

F32 = mybir.dt.float32
BF16 = mybir.dt.bfloat16

B, N, C, H = 16, 207, 64, 64
J = 3 * H                  # 192
ITEMS = B * N              # 3312
NCORES = 8
PER = ITEMS // NCORES      # 414
CHUNKS = [8, 16, 24, 32, 36, 36, 36, 36, 36, 36, 36, 36, 20, 14, 8, 4]  # 414
NCHUNK = len(CHUNKS)
GMAX = max(CHUNKS)
# weight DMA granularity: the early ramp chunks are shipped as two
# host-contiguous half-chunk blocks [c, item, j] so the first matmuls
# start at half-chunk latency; steady chunks go as single larger DMAs
def _halves(k, G):
    if k >= 4 or G < 16:
        return [(0, G)]
    G2 = G // 2
    return [(0, G2), (G2, G)]

AF = mybir.ActivationFunctionType


def build_nc():
    nc = bacc.Bacc(None)
    # x|h transposed to [c/h, item] in bf16 (the PE moving columns)
    xh_d = nc.declare_dram_parameter("xh", [128, PER], BF16, isOutput=False)
    # h transposed, f32, for the epilogue (loaded into rows 64:128)
    hf_d = nc.declare_dram_parameter("hf", [64, PER], F32, isOutput=False)
    brz_d = nc.declare_dram_parameter("brz", [128, PER], F32, isOutput=False)
    bc_d = nc.declare_dram_parameter("bc", [64, PER], F32, isOutput=False)
    # weights, host-pre-transposed per chunk to [c, item, j] (flattened)
    w_d = nc.declare_dram_parameter("wxh", [PER * 2 * C * J], BF16,
                                    isOutput=False)
    # output stays j-major [H, items]; the host transposes (trivial)
    out_d = nc.declare_dram_parameter("out", [H, PER], F32, isOutput=True)

    with tile.TileContext(nc) as tc:
        with (
            tc.tile_pool(name="const", bufs=1) as cpool,
            tc.tile_pool(name="w", bufs=5) as wpool,
            tc.tile_pool(name="act", bufs=2) as apool,
            tc.tile_pool(name="carry", bufs=4) as xpool,
            tc.tile_pool(name="prz", bufs=3, space="PSUM") as prz_pool,
            tc.tile_pool(name="pc", bufs=3, space="PSUM") as pc_pool,
        ):
            # ---- persistent activations / bias -------------------------
            # HWDGE (sync/scalar) loads issued BEFORE any weight DMA: they
            # land in ~1us, so the first matmul isn't gated on a SWDGE
            # transfer stuck behind the weight stream.
            xh = cpool.tile([128, PER], BF16)
            nc.sync.dma_start(out=xh[:], in_=xh_d[:])
            hf = cpool.tile([128, PER], F32)      # rows 64:128 = h (f32)
            nc.scalar.dma_start(out=hf[64:128, :], in_=hf_d[:])
            brz = cpool.tile([128, PER], F32)
            nc.sync.dma_start(out=brz[:], in_=brz_d[:])
            bc = cpool.tile([64, PER], F32)
            nc.scalar.dma_start(out=bc[:], in_=bc_d[:])

            # persistent c-pass moving tile: rows 0:64 (= x, bf16) filled
            # once; each chunk's epilogue writes rows 64:128 (= r*h)
            rhs2_all = cpool.tile([128, PER], BF16)
            nc.vector.tensor_copy(rhs2_all[0:64, :], xh[0:64, :])
            # persistent j-major output accumulator (rows 64:128 = h_new)
            hn_all = cpool.tile([128, PER], F32)

            starts = np.concatenate([[0], np.cumsum(CHUNKS)])[:-1]
            woffs = [int(s) * 2 * C * J for s in starts]

            # per-chunk state carried between pipeline stages
            wt = [None] * NCHUNK
            rz_ps = [None] * NCHUNK
            zs = [None] * NCHUNK

            def stage_load_rz(k):
                G = CHUNKS[k]
                s = int(starts[k])
                w = wpool.tile([128, GMAX, J], BF16, tag="w")
                wt[k] = w
                wq = nc.sync if k % 2 == 0 else nc.scalar
                # two half-chunk DMAs: the first half's matmuls can start
                # while the second half is still in flight
                off = woffs[k]
                for h0, h1 in _halves(k, G):
                    nh = 128 * (h1 - h0) * J
                    wq.dma_start(
                        out=w[:, h0:h1, :],
                        in_=w_d[off:off + nh].rearrange(
                            "(c g j) -> c g j", c=128, g=h1 - h0),
                    )
                    off += nh
                psum_rz = prz_pool.tile([128, GMAX], F32, tag="rz")
                rz_ps[k] = psum_rz
                for g in range(G):
                    nc.tensor.matmul(
                        psum_rz[:, g:g + 1],
                        w[:, g, 0:128],
                        xh[:, s + g:s + g + 1],
                        start=True, stop=True,
                    )

            def stage_mid(k):
                # epilogue 1: r, z, and the c-pass moving columns
                G = CHUNKS[k]
                s = int(starts[k])
                psum_rz = rz_ps[k]
                t_rz = apool.tile([128, GMAX], F32, tag="t_rz")
                nc.vector.tensor_add(t_rz[:, 0:G], psum_rz[:, 0:G],
                                     brz[:, s:s + G])
                rs = apool.tile([128, GMAX], F32, tag="rs")
                nc.scalar.activation(rs[64:128, 0:G], t_rz[0:64, 0:G],
                                     AF.Sigmoid)
                z = xpool.tile([128, GMAX], F32, tag="zs")
                zs[k] = z
                nc.scalar.activation(z[64:128, 0:G], t_rz[64:128, 0:G],
                                     AF.Sigmoid)
                nc.vector.tensor_mul(rhs2_all[64:128, s:s + G],
                                     rs[64:128, 0:G], hf[64:128, s:s + G])

            def stage_c(k):
                G = CHUNKS[k]
                s = int(starts[k])
                w = wt[k]
                psum_c = pc_pool.tile([64, GMAX], F32, tag="c")
                for g in range(G):
                    nc.tensor.matmul(
                        psum_c[0:64, g:g + 1],
                        w[:, g, 128:192],
                        rhs2_all[:, s + g:s + g + 1],
                        start=True, stop=True,
                    )
                wt[k] = None
                t_c = apool.tile([64, GMAX], F32, tag="t_c")
                nc.vector.tensor_add(t_c[0:64, 0:G], psum_c[0:64, 0:G],
                                     bc[:, s:s + G])
                hc = apool.tile([128, GMAX], F32, tag="hc")
                nc.scalar.activation(hc[64:128, 0:G], t_c[0:64, 0:G], AF.Tanh)
                # h_new = h + z * (hc - h)
                dd = apool.tile([128, GMAX], F32, tag="d")
                nc.vector.tensor_sub(dd[64:128, 0:G], hc[64:128, 0:G],
                                     hf[64:128, s:s + G])
                zd = apool.tile([128, GMAX], F32, tag="zd")
                nc.vector.tensor_mul(zd[64:128, 0:G], zs[k][64:128, 0:G],
                                     dd[64:128, 0:G])
                nc.vector.tensor_add(hn_all[64:128, s:s + G],
                                     zd[64:128, 0:G], hf[64:128, s:s + G])
                zs[k] = None
                rz_ps[k] = None

            # software pipeline, depth 2: rz(k) and rz(k+1) are traced
            # before c(k), so the PE never waits on the DVE/ACT chain
            # that produces chunk k's r*h moving columns
            for k in range(NCHUNK + 1):
                if k < NCHUNK:
                    stage_load_rz(k)
                    stage_mid(k)
                if k >= 1:
                    stage_c(k - 1)

            # one j-major store of the whole output (host transposes)
            nc.gpsimd.dma_start(out=out_d[:], in_=hn_all[64:128, :])

    nc.compile()
    return nc


_CACHE = {}


def _get_nc():
    if "nc" not in _CACHE:
        _CACHE["nc"] = build_nc()
    return _CACHE["nc"]


def _shards(x, state, Wx, Wh, b):
    import ml_dtypes
    bf16 = ml_dtypes.bfloat16
    x2 = np.asarray(x, np.float32).reshape(ITEMS, C)
    h2 = np.asarray(state, np.float32).reshape(ITEMS, H)
    b2 = np.asarray(b, np.float32).reshape(ITEMS, J)
    xh2 = np.concatenate([x2, h2], axis=1).astype(bf16)   # [ITEMS, 128]
    wx2 = np.asarray(Wx).reshape(ITEMS, C, J)
    wh2 = np.asarray(Wh).reshape(ITEMS, H, J)
    w2 = np.concatenate([wx2, wh2], axis=1).astype(bf16)  # [ITEMS, 128, J]
    w2 = w2.reshape(NCORES, PER, 2 * C, J)
    maps = []
    for i in range(NCORES):
        sl = slice(i * PER, (i + 1) * PER)
        blocks = []
        s = 0
        for k, G in enumerate(CHUNKS):
            for h0, h1 in _halves(k, G):
                blocks.append(w2[i, s + h0:s + h1].transpose(1, 0, 2).ravel())
            s += G
        maps.append({
            "xh": np.ascontiguousarray(xh2[sl].T),
            "hf": np.ascontiguousarray(h2[sl].T),
            "brz": np.ascontiguousarray(b2[sl, 0:128].T),
            "bc": np.ascontiguousarray(b2[sl, 128:192].T),
            "wxh": np.concatenate(blocks),
        })
    return maps


def kernel(x, state, Wx, Wh, b, _trace=False, **_ignored):
    nc = _get_nc()
    in_maps = _shards(x, state, Wx, Wh, b)
    res = run_bass_kernel_spmd(nc, in_maps, list(range(NCORES)), trace=_trace)
    out = np.concatenate(
        [res.results[i]["out"].T for i in range(NCORES)], axis=0)
    ret = out.reshape(B, N, 1, H).astype(np.float32)
    if _trace:
        return ret, res
    return ret


# revision 31
# speedup vs baseline: 1.0810x; 1.0810x over previous
"""Trainium2 Bass kernel for batched per-item GRU cell.

Problem: nn_GRU_Cell — B=16, N=207 independent items, each with its own
C=64 -> 3H=192 weight matrices (Wx, Wh).  All ops are per-(b,n):

    xW          = x @ Wx                      [1, 192]
    r           = sigmoid(xW_r + h @ Wh_r + b_r)
    z           = sigmoid(xW_z + h @ Wh_z + b_z)
    hc          = tanh  (xW_c + (r*h) @ Wh_c + b_c)
    h_new       = (1 - z) * h + z * hc

Strategy (per core, items sharded 3312 -> 8 x 414):
  * Weights dominate DMA (bf16: ~20.6MB/core).  Stream once, chunked.
  * Per item, weights are the PE *stationary* operand, K-stacked:
      S_rz = [Wx[:, 0:128] ; Wh[:, 0:128]]  (K=128, M=128)
      S_c  = [Wx[:,128:192]; Wh[:,128:192]] (K=128, M=64)
    moving operand is one bf16 column per item:
      rz-pass: [x ; h]    -> psum_rz[:, item]
      c-pass : [x ; r*h]  -> psum_c[0:64, item]   (folds xW_c in)
  * bf16 weights + moving columns: single-instruction LDW/MM (no
    fp32 LOW/HIGH doubling), FWL weight loads, half the DMA bytes.
    Max rel err vs fp32 reference ~1.8e-3.
  * x/h/bias arrive host-pre-transposed to [j, items]: no on-chip
    transposes; they're loaded once into persistent SBUF tiles.
  * Software pipeline: chunk k's rz matmuls are traced before chunk
    k-1's c matmuls, so the PE always has independent work while the
    DVE/ACT epilogue of the previous chunk produces r*h.
  * Chunk sizes ramp up (first w-DMA gates the first matmul) and the
    last chunk is short (post-PE drain tail).
"""

import numpy as np

import concourse.bass as bass
import concourse.mybir as mybir
import concourse.tile as tile
from concourse import bacc
from concourse.bass_utils import run_bass_kernel_spmd

F32 = mybir.dt.float32
BF16 = mybir.dt.bfloat16

B, N, C, H = 16, 207, 64, 64
J = 3 * H                  # 192
ITEMS = B * N              # 3312
NCORES = 8
PER = ITEMS // NCORES      # 414
CHUNKS = [8, 16, 24, 32, 36, 36, 36, 36, 36, 36, 36, 36, 20, 14, 8, 4]  # 414
NCHUNK = len(CHUNKS)
GMAX = max(CHUNKS)
# weight DMA granularity: the early ramp chunks are shipped as two
# host-contiguous half-chunk blocks [c, item, j] so the first matmuls
# start at half-chunk latency; steady chunks go as single larger DMAs
def _halves(k, G):
    if G < 24:
        return [(0, G)]
    G2 = G // 2
    return [(0, G2), (G2, G)]

AF = mybir.ActivationFunctionType


def build_nc():
    nc = bacc.Bacc(None)
    # x|h transposed to [c/h, item] in bf16 (the PE moving columns)
    xh_d = nc.declare_dram_parameter("xh", [128, PER], BF16, isOutput=False)
    # h transposed, f32, for the epilogue (loaded into rows 64:128)
    hf_d = nc.declare_dram_parameter("hf", [64, PER], F32, isOutput=False)
    brz_d = nc.declare_dram_parameter("brz", [128, PER], F32, isOutput=False)
    bc_d = nc.declare_dram_parameter("bc", [64, PER], F32, isOutput=False)
    # weights, host-pre-transposed per chunk to [c, item, j] (flattened)
    w_d = nc.declare_dram_parameter("wxh", [PER * 2 * C * J], BF16,
                                    isOutput=False)
    # output stays j-major [H, items]; the host transposes (trivial)
    out_d = nc.declare_dram_parameter("out", [H, PER], F32, isOutput=True)

    with tile.TileContext(nc) as tc:
        with (
            tc.tile_pool(name="const", bufs=1) as cpool,
            tc.tile_pool(name="w", bufs=5) as wpool,
            tc.tile_pool(name="act", bufs=2) as apool,
            tc.tile_pool(name="carry", bufs=4) as xpool,
            tc.tile_pool(name="prz", bufs=3, space="PSUM") as prz_pool,
            tc.tile_pool(name="pc", bufs=3, space="PSUM") as pc_pool,
        ):
            # ---- persistent activations / bias -------------------------
            # HWDGE (sync/scalar) loads issued BEFORE any weight DMA: they
            # land in ~1us, so the first matmul isn't gated on a SWDGE
            # transfer stuck behind the weight stream.
            xh = cpool.tile([128, PER], BF16)
            nc.sync.dma_start(out=xh[:], in_=xh_d[:])
            hf = cpool.tile([128, PER], F32)      # rows 64:128 = h (f32)
            nc.scalar.dma_start(out=hf[64:128, :], in_=hf_d[:])
            brz = cpool.tile([128, PER], F32)
            nc.sync.dma_start(out=brz[:], in_=brz_d[:])
            bc = cpool.tile([64, PER], F32)
            nc.scalar.dma_start(out=bc[:], in_=bc_d[:])

            # persistent c-pass moving tile: rows 0:64 (= x, bf16) filled
            # once; each chunk's epilogue writes rows 64:128 (= r*h)
            rhs2_all = cpool.tile([128, PER], BF16)
            nc.vector.tensor_copy(rhs2_all[0:64, :], xh[0:64, :])
            # persistent j-major output accumulator (rows 64:128 = h_new)
            hn_all = cpool.tile([128, PER], F32)

            starts = np.concatenate([[0], np.cumsum(CHUNKS)])[:-1]
            woffs = [int(s) * 2 * C * J for s in starts]

            # per-chunk state carried between pipeline stages
            wt = [None] * NCHUNK
            rz_ps = [None] * NCHUNK
            zs = [None] * NCHUNK

            def stage_load_rz(k):
                G = CHUNKS[k]
                s = int(starts[k])
                w = wpool.tile([128, GMAX, J], BF16, tag="w")
                wt[k] = w
                wq = nc.sync if k % 2 == 0 else nc.scalar
                # two half-chunk DMAs: the first half's matmuls can start
                # while the second half is still in flight
                off = woffs[k]
                for h0, h1 in _halves(k, G):
                    nh = 128 * (h1 - h0) * J
                    wq.dma_start(
                        out=w[:, h0:h1, :],
                        in_=w_d[off:off + nh].rearrange(
                            "(c g j) -> c g j", c=128, g=h1 - h0),
                    )
                    off += nh
                psum_rz = prz_pool.tile([128, GMAX], F32, tag="rz")
                rz_ps[k] = psum_rz
                for g in range(G):
                    nc.tensor.matmul(
                        psum_rz[:, g:g + 1],
                        w[:, g, 0:128],
                        xh[:, s + g:s + g + 1],
                        start=True, stop=True,
                    )

            def stage_mid(k):
                # epilogue 1: r, z, and the c-pass moving columns
                G = CHUNKS[k]
                s = int(starts[k])
                psum_rz = rz_ps[k]
                t_rz = apool.tile([128, GMAX], F32, tag="t_rz")
                nc.vector.tensor_add(t_rz[:, 0:G], psum_rz[:, 0:G],
                                     brz[:, s:s + G])
                rs = apool.tile([128, GMAX], F32, tag="rs")
                nc.scalar.activation(rs[64:128, 0:G], t_rz[0:64, 0:G],
                                     AF.Sigmoid)
                z = xpool.tile([128, GMAX], F32, tag="zs")
                zs[k] = z
                nc.scalar.activation(z[64:128, 0:G], t_rz[64:128, 0:G],
                                     AF.Sigmoid)
                nc.vector.tensor_mul(rhs2_all[64:128, s:s + G],
                                     rs[64:128, 0:G], hf[64:128, s:s + G])

            def stage_c(k):
                G = CHUNKS[k]
                s = int(starts[k])
                w = wt[k]
                psum_c = pc_pool.tile([64, GMAX], F32, tag="c")
                for g in range(G):
                    nc.tensor.matmul(
                        psum_c[0:64, g:g + 1],
                        w[:, g, 128:192],
                        rhs2_all[:, s + g:s + g + 1],
                        start=True, stop=True,
                    )
                wt[k] = None
                t_c = apool.tile([64, GMAX], F32, tag="t_c")
                nc.vector.tensor_add(t_c[0:64, 0:G], psum_c[0:64, 0:G],
                                     bc[:, s:s + G])
                hc = apool.tile([128, GMAX], F32, tag="hc")
                nc.scalar.activation(hc[64:128, 0:G], t_c[0:64, 0:G], AF.Tanh)
                # h_new = h + z * (hc - h)
                dd = apool.tile([128, GMAX], F32, tag="d")
                nc.vector.tensor_sub(dd[64:128, 0:G], hc[64:128, 0:G],
                                     hf[64:128, s:s + G])
                zd = apool.tile([128, GMAX], F32, tag="zd")
                nc.vector.tensor_mul(zd[64:128, 0:G], zs[k][64:128, 0:G],
                                     dd[64:128, 0:G])
                nc.vector.tensor_add(hn_all[64:128, s:s + G],
                                     zd[64:128, 0:G], hf[64:128, s:s + G])
                zs[k] = None
                rz_ps[k] = None

            # software pipeline, depth 2: rz(k) and rz(k+1) are traced
            # before c(k), so the PE never waits on the DVE/ACT chain
            # that produces chunk k's r*h moving columns
            for k in range(NCHUNK + 2):
                if k < NCHUNK:
                    stage_load_rz(k)
                    stage_mid(k)
                if k >= 2:
                    stage_c(k - 2)

            # one j-major store of the whole output (host transposes)
            nc.gpsimd.dma_start(out=out_d[:], in_=hn_all[64:128, :])

    nc.compile()
    return nc


_CACHE = {}


def _get_nc():
    if "nc" not in _CACHE:
        _CACHE["nc"] = build_nc()
    return _CACHE["nc"]


def _shards(x, state, Wx, Wh, b):
    import ml_dtypes
    bf16 = ml_dtypes.bfloat16
    x2 = np.asarray(x, np.float32).reshape(ITEMS, C)
    h2 = np.asarray(state, np.float32).reshape(ITEMS, H)
    b2 = np.asarray(b, np.float32).reshape(ITEMS, J)
    xh2 = np.concatenate([x2, h2], axis=1).astype(bf16)   # [ITEMS, 128]
    wx2 = np.asarray(Wx).reshape(ITEMS, C, J)
    wh2 = np.asarray(Wh).reshape(ITEMS, H, J)
    w2 = np.concatenate([wx2, wh2], axis=1).astype(bf16)  # [ITEMS, 128, J]
    w2 = w2.reshape(NCORES, PER, 2 * C, J)
    maps = []
    for i in range(NCORES):
        sl = slice(i * PER, (i + 1) * PER)
        blocks = []
        s = 0
        for k, G in enumerate(CHUNKS):
            for h0, h1 in _halves(k, G):
                blocks.append(w2[i, s + h0:s + h1].transpose(1, 0, 2).ravel())
            s += G
        maps.append({
            "xh": np.ascontiguousarray(xh2[sl].T),
            "hf": np.ascontiguousarray(h2[sl].T),
            "brz": np.ascontiguousarray(b2[sl, 0:128].T),
            "bc": np.ascontiguousarray(b2[sl, 128:192].T),
            "wxh": np.concatenate(blocks),
        })
    return maps


def kernel(x, state, Wx, Wh, b, _trace=False, **_ignored):
    nc = _get_nc()
    in_maps = _shards(x, state, Wx, Wh, b)
    res = run_bass_kernel_spmd(nc, in_maps, list(range(NCORES)), trace=_trace)
    out = np.concatenate(
        [res.results[i]["out"].T for i in range(NCORES)], axis=0)
    ret = out.reshape(B, N, 1, H).astype(np.float32)
    if _trace:
        return ret, res
    return ret
